# revision 4
# baseline (speedup 1.0000x reference)
import math
import numpy as np

# ---- problem constants (hardcoded per spec) ----
S, K, O, H = 64, 16, 32, 30   # state_dim, act_dim, obs_dim, horizon
T, N = 200, 512               # seq len, batch
EPS = 1e-6
LOG2PI = float(np.log(2.0 * np.pi))
NUM_PARAMS = S * O * 2 + K * S * S + S + S + 1
NDEV = 8
NL = N // NDEV                # 64 batch elements per core

_fn = None


def _build():
    import jax
    import jax.numpy as jnp
    from jax.scipy.special import logsumexp

    P = jax.lax.Precision.HIGHEST
    lgam = jnp.asarray([math.lgamma(h + 1.0) for h in range(H)], dtype=jnp.float32)
    hh = jnp.arange(H, dtype=jnp.float32)

    def shard_fn(o, a, theta):
        # o [T, NL, O] f32, a [T, NL] i32, theta [NL, NUM_PARAMS] f32
        s1 = S * O * 2
        s2 = s1 + K * S * S
        s3 = s2 + S
        s4 = s3 + S
        tA = theta[:, :s1]
        tB = theta[:, s1:s2]
        tC = theta[:, s2:s3]
        tD = theta[:, s3:s4]
        ttau = theta[:, s4:]
        A_mean, A_lv = jnp.split(tA, 2, axis=-1)
        A_mean = A_mean.reshape(NL, S, O)
        # softplus/sigmoid written in exp/log form: walrus's lower_act
        # crashes (calculateBestSets) on the fused softplus primitive here
        alv = A_lv.reshape(NL, S, O)
        # softplus(x) = -log(sigmoid(-x)); Sigmoid+Log both have ACT table
        # sets, unlike the fused Softplus the tensorizer otherwise emits
        A_std = -jnp.log(jax.nn.sigmoid(-alv)) + EPS
        Bm = jax.nn.softmax(tB.reshape(NL, K, S, S), axis=-1)
        C = jax.nn.softmax(tC.reshape(NL, 1, S), axis=-1)
        D = jax.nn.softmax(tD.reshape(NL, S), axis=-1)
        tau = (2.0 * H) / (1.0 + jnp.exp(-ttau)) + 1.0  # sigmoid(t)*2H + 1, [NL,1]

        # ---- reward r[n,k,i] ----
        ent = jnp.sum(0.5 + 0.5 * LOG2PI + jnp.log(A_std), axis=-1)            # [NL,S]
        kl = jnp.sum(Bm * (jnp.log(Bm + EPS) - jnp.log(C[:, None] + EPS)), -1)  # [NL,K,S]
        eh = jnp.einsum('nkij,nj->nki', Bm, ent, precision=P)
        r = -kl - eh

        # ---- finite-horizon value iteration ----
        def vi_step(qt, _):
            v = logsumexp(qt, axis=-2)                    # [NL,S]
            qn = r + jnp.einsum('nkij,nj->nki', Bm, v, precision=P)
            return qn, qn
        _, q_rest = jax.lax.scan(vi_step, r, None, length=H - 1)
        q = jnp.concatenate([r[None], q_rest], axis=0)    # [H,NL,K,S]

        # ---- truncated poisson horizon weights ----
        logp_h = hh[None, :] * jnp.log(tau) - tau - lgam[None, :]
        pdf = jnp.exp(logp_h)
        pdf = pdf / jnp.sum(pdf, axis=-1, keepdims=True)  # [NL,H]

        # ---- precompute observation loglik sums for all t (batched matmuls) ----
        # sum_o lp[t,n,s] = sum_o [ -0.5((o-mu)/sig)^2 - log sig - 0.5*log2pi ]
        inv2 = 1.0 / (A_std * A_std)                      # [NL,S,O]
        W1 = A_mean * inv2
        c0 = jnp.sum(-0.5 * A_mean * A_mean * inv2 - jnp.log(A_std)
                     - 0.5 * LOG2PI, axis=-1)             # [NL,S]
        sumlp = (jnp.einsum('tno,nso->tns', o, W1, precision=P)
                 - 0.5 * jnp.einsum('tno,nso->tns', o * o, inv2, precision=P)
                 + c0[None])                              # [T,NL,S]

        # ---- belief recursion over t (policy decoupled below) ----
        batch_idx = jnp.arange(NL)

        def time_step(b, inp):
            a_t, slp = inp
            B_a = Bm[batch_idx, a_t]                      # [NL,S,S]
            s = jnp.einsum('nij,ni->nj', B_a, b, precision=P)
            joint = jnp.log(s + EPS) + slp                # [NL,S]
            b_next = jax.nn.softmax(joint, axis=-1)
            logp_o = logsumexp(joint, axis=-1)
            return b_next, (b, logp_o)

        _, (b_seq, logp_o) = jax.lax.scan(time_step, D, (a, sumlp))

        # ---- policy, batched over all t ----
        qb = jnp.einsum('tni,hnki->thnk', b_seq, q, precision=P)  # [T,H,NL,K]
        pi_h = jax.nn.softmax(qb, axis=-1)
        pi = jnp.einsum('thnk,nh->tnk', pi_h, pdf, precision=P)   # [T,NL,K]
        return pi, b_seq, logp_o

    return jax.pmap(shard_fn)


def kernel(o, a, theta):
    global _fn
    if _fn is None:
        _fn = _build()
    o = np.asarray(o, dtype=np.float32)
    a = np.asarray(a)
    theta = np.asarray(theta, dtype=np.float32)
    a_dt = a.dtype  # preserve index dtype semantics (values < K fit any int)

    o_sh = np.ascontiguousarray(o.reshape(T, NDEV, NL, O).transpose(1, 0, 2, 3))
    a_sh = np.ascontiguousarray(a.astype(np.int32).reshape(T, NDEV, NL).transpose(1, 0, 2))
    th_sh = np.ascontiguousarray(theta.reshape(NDEV, NL, NUM_PARAMS))

    pi, b_seq, logp_o = _fn(o_sh, a_sh, th_sh)
    pi = np.asarray(pi).transpose(1, 0, 2, 3).reshape(T, N, K)
    b_seq = np.asarray(b_seq).transpose(1, 0, 2, 3).reshape(T, N, S)
    logp_o = np.asarray(logp_o).transpose(1, 0, 2).reshape(T, N)
    return pi, b_seq, logp_o
